# revision 4
# baseline (speedup 1.0000x reference)
"""Binary conv2d (XNOR-style) + per-channel scale for Trainium2.

y = conv2d(sign(x), sign(w), stride=1, pad=1) * scale[oc]

Strategy: data-parallel over batch across 8 NeuronCores (4 images each).
Per core: binarize x into a zero-padded 58x58 bf16 image laid out
[ic_part=128, spatial]; the 3x3 conv over 256 in-channels becomes 18
accumulating matmuls (2 ic blocks x 9 taps) into a PSUM tile per
8-output-row chunk, using shifted windows of the padded image as the
moving operand.  All matmul inputs are exactly representable (+-1/0 in
bf16) and PSUM accumulates in fp32, so the conv result is the exact
integer; the per-channel scale is applied in fp32 during the PSUM->SBUF
copy, making the result bit-identical to the fp32 reference.
"""

import numpy as np

N_CORES = 8
IMGS = 4  # images per core
IC = 256
OC = 256
H = W = 56
WPAD = 58  # 56 + 2 zero pad columns/rows
XPAD_F = 3368  # padded image free size: 58*58=3364, +4 pad (max read 3366)
ROWS = 8  # output rows per PSUM tile
NFREE = ROWS * WPAD  # 464 <= 512 (PSUM bank limit)
NCHUNK = H // ROWS  # 7

_cache = {}


def _install_drain_patch():
    """This walrus build rejects >2 sync-waits on an InstDrain; Tile's
    kernel-tail drain carries one wait per pending proc.  Split it into
    one drain per proc (each with <=1 wait)."""
    import concourse.tile as _tile
    from concourse.vector_clock import ScopedClock, VectorClock

    if getattr(_tile.TileContext, "_drain_split_patch", False):
        return

    def _drain_and_barrier(self, tick_clock, wait_clock):
        nc = self.nc
        gclock = tick_clock.global_clock
        n = len(gclock)
        for p in range(n):
            t = gclock[p]
            if t <= 0:
                continue
            vec = [0] * n
            vec[p] = t
            d = nc.sync.drain()
            wait_clock.add_sem_waits(d.ins, ScopedClock({None: VectorClock(vec)}))
        nc.all_engine_barrier()
        assert self.sems is not None
        popped = nc._tile_sem_poison_stack.pop()
        assert popped is self._sem_poison
        nc.clear_and_free_semaphores(list(self.sems.allocated().values()))
        nc.all_engine_barrier()

    _tile.TileContext._drain_and_barrier = _drain_and_barrier
    _tile.TileContext._drain_split_patch = True


def _split_excess_waits(nc, maxw=1):
    """Same walrus limitation as the drain: instructions only accept ~2
    sync-waits.  Hoist any excess onto same-engine NoOps inserted just
    before the instruction (engine streams are in-order, so a preceding
    NoOp carrying the waits is equivalent)."""
    import concourse.mybir as mybir

    n_split = 0
    for f in nc.m.functions:
        for bb in f.blocks:
            out = []
            for ins in bb.instructions:
                si = ins.sync_info
                if si and si.on_wait and len(si.on_wait) > maxw:
                    waits = list(si.on_wait)
                    excess, keep = waits[:-maxw], waits[-maxw:]
                    for i in range(0, len(excess), maxw):
                        nop = mybir.InstNoOp(
                            name=f"{ins.name}_waitsplit{i}",
                            engine=ins.engine,
                            ins=[],
                            outs=[],
                            sync_info=mybir.SyncInfo(
                                on_wait=excess[i : i + maxw], on_update=[]
                            ),
                        )
                        out.append(nop)
                    si.on_wait = keep
                    n_split += 1
                out.append(ins)
            bb.instructions = out
    return n_split


def build_nc():
    import concourse.bass as bass
    import concourse.mybir as mybir
    from concourse.tile import TileContext

    _install_drain_patch()

    f32 = mybir.dt.float32
    bf16 = mybir.dt.bfloat16
    Copy = mybir.ActivationFunctionType.Copy

    nc = bass.Bass()
    x = nc.declare_dram_parameter("x", [IMGS, IC, H, W], f32, isOutput=False)
    wt = nc.declare_dram_parameter("wt", [3, 3, IC, OC], f32, isOutput=False)
    scale = nc.declare_dram_parameter("scale", [OC], f32, isOutput=False)
    y = nc.declare_dram_parameter("y", [IMGS, OC, H, W], f32, isOutput=True)

    with TileContext(nc) as tc:
        with (
            tc.tile_pool(name="const", bufs=1) as cpool,
            tc.tile_pool(name="xin", bufs=3) as xin_pool,
            tc.tile_pool(name="outp", bufs=3) as out_pool,
            tc.tile_pool(name="psum", bufs=8, space="PSUM") as psum_pool,
        ):
            # --- weights: [3,3,ic,oc] f32 -> sign -> bf16 [ic128, 18, oc256]
            wf = cpool.tile([128, 18, OC], f32)
            for kh in range(3):
                for kw in range(3):
                    for icb in range(2):
                        s = (kh * 3 + kw) * 2 + icb
                        nc.sync.dma_start(
                            out=wf[:, s, :],
                            in_=wt[kh, kw, icb * 128 : (icb + 1) * 128, :],
                        )
            wb = cpool.tile([128, 18, OC], bf16)
            nc.scalar.sign(wb[:], wf[:])

            # --- per-oc scale -> [p, ocb]
            sc = cpool.tile([128, 2], f32)
            nc.sync.dma_start(out=sc[:], in_=scale.rearrange("(b p) -> p b", p=128))

            # --- padded, binarized activations: slot j = n*2+icb
            xp = cpool.tile([128, IMGS * 2, XPAD_F], bf16)
            nc.vector.memset(xp[:], 0.0)
            for n in range(IMGS):
                for icb in range(2):
                    j = n * 2 + icb
                    xin = xin_pool.tile([128, H, W], f32)
                    nc.sync.dma_start(
                        out=xin[:], in_=x[n, icb * 128 : (icb + 1) * 128, :, :]
                    )
                    dst = (
                        xp[:, j, WPAD + 1 : WPAD + 1 + H * WPAD]
                        .rearrange("p (h w) -> p h w", w=WPAD)[:, :, 0:W]
                    )
                    nc.scalar.sign(dst, xin[:])

            # --- conv as 18 accumulating matmuls per 8-row chunk
            for n in range(IMGS):
                for ocb in range(2):
                    out_sb = out_pool.tile([128, H, W], f32)
                    for c in range(NCHUNK):
                        ps = psum_pool.tile([128, NFREE], f32)
                        s = 0
                        for icb in range(2):
                            xpj = xp[:, n * 2 + icb, :]
                            for t in range(9):
                                kh, kw = divmod(t, 3)
                                off = c * ROWS * WPAD + kh * WPAD + kw
                                nc.tensor.matmul(
                                    ps[:],
                                    wb[:, (kh * 3 + kw) * 2 + icb, ocb * 128 : (ocb + 1) * 128],
                                    xpj[:, off : off + NFREE],
                                    start=(s == 0),
                                    stop=(s == 17),
                                )
                                s += 1
                        src = ps.rearrange("p (h w) -> p h w", w=WPAD)[:, :, 0:W]
                        nc.scalar.activation(
                            out_sb[:, c * ROWS : (c + 1) * ROWS, :],
                            src,
                            Copy,
                            scale=sc[:, ocb : ocb + 1],
                        )
                    nc.sync.dma_start(
                        out=y[n, ocb * 128 : (ocb + 1) * 128, :, :], in_=out_sb[:]
                    )

    _split_excess_waits(nc)
    return nc


def _get_nc():
    if "nc" not in _cache:
        _cache["nc"] = build_nc()
    return _cache["nc"]


def run(inputs, trace=False, trace_cores=None):
    from concourse.bass_utils import run_bass_kernel_spmd

    x = np.asarray(inputs["x"])
    weight = np.asarray(inputs["weight"])
    scale = np.asarray(inputs["scale"])

    # (kh, kw, ic, oc) layout so each tap's [ic, oc] block is contiguous
    wt = np.ascontiguousarray(weight.transpose(2, 3, 1, 0)).astype(np.float32)

    in_maps = [
        {"x": x[i * IMGS : (i + 1) * IMGS], "wt": wt, "scale": scale}
        for i in range(N_CORES)
    ]
    res = run_bass_kernel_spmd(
        _get_nc(),
        in_maps,
        core_ids=list(range(N_CORES)),
        trace=trace,
        trace_cores=trace_cores,
    )
    out = np.concatenate([res.results[i]["y"] for i in range(N_CORES)], axis=0)
    return out, res


def kernel(**inputs):
    out, _ = run(inputs, trace=False)
    return out


# revision 27
# speedup vs baseline: 1.7829x; 1.7829x over previous
"""Binary conv2d (XNOR-style) + per-channel scale for Trainium2.

y = conv2d(sign(x), sign(w), stride=1, pad=1) * scale[oc]

Data-parallel over batch across 8 NeuronCores (4 images each).  Per
core: binarize x into a zero-padded 58x58 fp8/bf16 image laid out
[ic_part=128, slot, spatial] (slot = image*2 + ic_block); the 3x3 conv
over 256 in-channels becomes accumulating matmuls into a PSUM tile per
8-output-row chunk, using shifted windows of the padded image as the
moving operand.  With fp8 DoubleRow both ic blocks contract in one
matmul (K=256).  All matmul inputs are exactly +-1/0 (representable in
fp8/bf16) and PSUM accumulates in fp32, so the conv result is the exact
integer; the per-channel scale is applied in fp32 during the PSUM->SBUF
copy, making the result bit-identical to the fp32 reference.
"""

import numpy as np

N_CORES = 8
IMGS = 4  # images per core
IC = 256
OC = 256
H = W = 56
WPAD = 58  # 56 + 2 zero pad columns/rows
XPAD_F = 3376  # padded image free size: 58*58=3364 -> pad to mult of 16
ROWS = 8  # output rows per PSUM tile
NFREE = ROWS * WPAD  # 464 <= 512 (PSUM bank limit)
NCHUNK = H // ROWS  # 7

USE_FP8 = True

_cache = {}


def _install_drain_patch():
    """This walrus build rejects >1 sync-wait on ctrl-type instructions;
    Tile's kernel-tail drain carries one wait per pending proc.  Split it
    into one drain per proc (each with <=1 wait)."""
    import concourse.tile as _tile
    from concourse.vector_clock import ScopedClock, VectorClock

    if getattr(_tile.TileContext, "_drain_split_patch", False):
        return

    def _drain_and_barrier(self, tick_clock, wait_clock):
        # Drains AND the sem clears both run on the Pool engine (the
        # framework's clear_and_free_semaphores emits gpsimd dma_reset/
        # sem_clear), so the first all-engine barrier of the stock tail is
        # unnecessary: once Pool's drains observe every proc's final tick,
        # all sem increments have retired and Pool may clear immediately.
        nc = self.nc
        gclock = tick_clock.global_clock
        n = len(gclock)
        for p in range(n):
            t = gclock[p]
            if t <= 0:
                continue
            vec = [0] * n
            vec[p] = t
            d = nc.gpsimd.drain()
            wait_clock.add_sem_waits(d.ins, ScopedClock({None: VectorClock(vec)}))
        assert self.sems is not None
        popped = nc._tile_sem_poison_stack.pop()
        assert popped is self._sem_poison
        nc.clear_and_free_semaphores(list(self.sems.allocated().values()))
        # No final all-engine barrier: every other engine's stream simply
        # ends, NRT completion waits for all queues anyway, and the next
        # invocation cannot start before this one fully retires.

    _tile.TileContext._drain_and_barrier = _drain_and_barrier
    _tile.TileContext._drain_split_patch = True


def _split_excess_waits(nc, maxw=1):
    """Same walrus limitation: hoist excess sync-waits onto same-engine
    NoOps inserted just before the instruction (engine streams are
    in-order, so a preceding NoOp carrying the waits is equivalent)."""
    import concourse.mybir as mybir

    n_split = 0
    for f in nc.m.functions:
        for bb in f.blocks:
            out = []
            for ins in bb.instructions:
                si = ins.sync_info
                if si and si.on_wait and len(si.on_wait) > maxw:
                    waits = list(si.on_wait)
                    excess, keep = waits[:-maxw], waits[-maxw:]
                    for i in range(0, len(excess), maxw):
                        nop = mybir.InstNoOp(
                            name=f"{ins.name}_waitsplit{i}",
                            engine=ins.engine,
                            ins=[],
                            outs=[],
                            sync_info=mybir.SyncInfo(
                                on_wait=excess[i : i + maxw], on_update=[]
                            ),
                        )
                        out.append(nop)
                    si.on_wait = keep
                    n_split += 1
                out.append(ins)
            bb.instructions = out
    return n_split


def build_nc():
    import concourse.bass as bass
    import concourse.mybir as mybir
    from concourse.tile import TileContext

    _install_drain_patch()

    f32 = mybir.dt.float32
    bdt = mybir.dt.float8e4 if USE_FP8 else mybir.dt.bfloat16
    Copy = mybir.ActivationFunctionType.Copy

    nc = bass.Bass()
    x = nc.declare_dram_parameter("x", [IMGS, IC, H, W], f32, isOutput=False)
    wt = nc.declare_dram_parameter("wt", [3, 3, IC, OC], f32, isOutput=False)
    scale = nc.declare_dram_parameter("scale", [OC], f32, isOutput=False)
    y = nc.declare_dram_parameter("y", [IMGS, OC, H, W], f32, isOutput=True)

    with TileContext(nc) as tc:
        with (
            tc.tile_pool(name="const", bufs=1) as cpool,
            tc.tile_pool(name="xin", bufs=3) as xin_pool,
            tc.tile_pool(name="outp", bufs=4) as out_pool,
            tc.tile_pool(name="psum", bufs=8, space="PSUM") as psum_pool,
        ):
            # --- DMA issue serializes ~0.6us each on the SP queue and HBM
            # delivery follows issue order.  The t=0 weight tap goes first
            # (tiny, unblocks the first LDWEIGHTS), then the image-0 loads
            # that gate the first matmul, then the remaining taps.
            # NOTE: a DoubleRow rhs AP spans both ic slots of an image, so
            # its dep range covers the WHOLE slot pair -- partial-image
            # binarization cannot unlock any matmul; sign whole images.
            wf = cpool.tile([128, 18, OC], f32)
            wb = cpool.tile([128, 18, OC], bdt)
            wsrc = wt.rearrange("a b (i p) f -> p (a b i) f", p=128)

            nc.sync.dma_start(out=wf[:, 0:2, :], in_=wsrc[:, 0:2, :])

            # x0 on the SP HWDGE ring, x1 on the Scalar-engine ring: rings
            # are FIFO per issuing engine, so this parallelizes the two
            # transfers that gate the first matmul.
            xp = cpool.tile([128, IMGS * 2, XPAD_F], bdt)
            xins = {}
            for j, eng in ((0, nc.sync), (1, nc.scalar)):
                xin = xin_pool.tile([128, H, W], f32, name=f"xin{j}", tag="xin")
                eng.dma_start(out=xin[:], in_=x[0, j * 128 : (j + 1) * 128, :, :])
                xins[j] = xin

            for t in range(1, 9):
                nc.sync.dma_start(
                    out=wf[:, 2 * t : 2 * t + 2, :], in_=wsrc[:, 2 * t : 2 * t + 2, :]
                )
            sc = cpool.tile([128, 2], f32)
            nc.sync.dma_start(out=sc[:], in_=scale.rearrange("(b p) -> p b", p=128))

            def pad_ring(j):
                # zero only the padding ring (interior is overwritten by
                # the sign): row 0 cols 0-56; the adjacent (col57 row r,
                # col0 row r+1) pairs; row 57 tail.
                xpj = xp[:, j, :]
                nc.vector.memset(xpj[:, 0:57], 0.0)
                pairs = xpj[:, 57 : 57 + 57 * WPAD].rearrange(
                    "p (r c) -> p r c", c=WPAD
                )[:, :, 0:2]
                nc.vector.memset(pairs, 0.0)
                nc.vector.memset(xpj[:, 57 * WPAD + 1 : XPAD_F], 0.0)

            def sign_dst(j, r0=0, r1=H):
                # destination for x rows [r0, r1) = padded rows [r0+1, r1+1)
                base = (r0 + 1) * WPAD + 1
                return (
                    xp[:, j, base : base + (r1 - r0) * WPAD]
                    .rearrange("p (h w) -> p h w", w=WPAD)[:, :, 0:W]
                )

            def dve_sign(j, xin, r0=0, r1=H, tg="tmp"):
                # sign() on the vector engine: clamp(x * 1e38, -1, 1).
                # Exact for fp32 normals (|x|*1e38 saturates past +-1) and
                # for +-0; frees ScalarE, which owns the other signs.
                tmp = xin_pool.tile(
                    [128, r1 - r0, W], f32, name=f"tmp{j}_{r0}", tag=tg
                )
                nc.vector.tensor_scalar(
                    tmp[:], xin[:], 1.0e38, -1.0,
                    op0=mybir.AluOpType.mult, op1=mybir.AluOpType.max,
                )
                nc.vector.tensor_scalar_min(sign_dst(j, r0, r1), tmp[:], 1.0)

            pad_ring(0)
            pad_ring(1)
            nc.scalar.sign(wb[:, 0:2, :], wf[:, 0:2, :])  # t=0 taps, early
            nc.scalar.sign(sign_dst(0), xins[0][:])  # ACT
            dve_sign(1, xins[1])  # DVE, parallel with ACT
            for t in range(1, 9):
                nc.scalar.sign(wb[:, 2 * t : 2 * t + 2, :], wf[:, 2 * t : 2 * t + 2, :])

            def load_image(n):
                # input loads ride the Scalar-engine HWDGE ring, leaving
                # the SP ring to the (larger) output-store stream.
                for icb in range(2):
                    j = n * 2 + icb
                    xin = xin_pool.tile([128, H, W], f32, name=f"xin{j}", tag="xin")
                    nc.scalar.dma_start(
                        out=xin[:], in_=x[n, icb * 128 : (icb + 1) * 128, :, :]
                    )
                    pad_ring(j)
                    if icb == 0:
                        nc.scalar.sign(sign_dst(j), xin[:])
                    else:
                        dve_sign(j, xin)

            def compute_image(n, subs=((0, NCHUNK),)):
                # tap-outer (weight-stationary) so consecutive matmuls hit
                # different PSUM banks (same-bank back-to-back accumulation
                # serializes the drain/fill overlap).  LDWEIGHTS overlaps
                # MATMUL via the PE dual weight buffer.  `subs` splits the
                # chunk range so the first subgroup can start before the
                # whole image is binarized (n=0) / drain earlier (n=3).
                for c0, c1 in subs:
                    for ocb in range(2):
                        psums = [
                            psum_pool.tile(
                                [128, NFREE], f32, name=f"ps{n}{ocb}{c}", tag="ps"
                            )
                            for c in range(c0, c1)
                        ]
                        for t in range(9):
                            kh, kw = divmod(t, 3)
                            if USE_FP8:
                                lhsT = wb[:, 2 * t : 2 * t + 2, ocb * 128 : (ocb + 1) * 128]
                                rhs_slot = xp[:, 2 * n : 2 * n + 2, :]
                                for c in range(c0, c1):
                                    off = c * ROWS * WPAD + kh * WPAD + kw
                                    nc.tensor.matmul(
                                        psums[c - c0][:],
                                        lhsT,
                                        rhs_slot[:, :, off : off + NFREE],
                                        start=(t == 0),
                                        stop=(t == 8),
                                        perf_mode=mybir.MatmulPerfMode.DoubleRow,
                                    )
                            else:
                                for icb in range(2):
                                    for c in range(c0, c1):
                                        off = c * ROWS * WPAD + kh * WPAD + kw
                                        nc.tensor.matmul(
                                            psums[c - c0][:],
                                            wb[:, 2 * t + icb, ocb * 128 : (ocb + 1) * 128],
                                            xp[:, n * 2 + icb, off : off + NFREE],
                                            start=(t == 0 and icb == 0),
                                            stop=(t == 8 and icb == 1),
                                        )
                        for c in range(c0, c1):
                            out_c = out_pool.tile([128, ROWS, W], f32)
                            src = psums[c - c0].rearrange("p (h w) -> p h w", w=WPAD)[
                                :, :, 0:W
                            ]
                            # alternate drain engine; both apply fp32 scale
                            if c % 2 == 1:
                                nc.scalar.activation(
                                    out_c[:], src, Copy, scale=sc[:, ocb : ocb + 1]
                                )
                            else:
                                nc.vector.tensor_scalar_mul(
                                    out_c[:], src, sc[:, ocb : ocb + 1]
                                )
                            nc.sync.dma_start(
                                out=y[n, ocb * 128 : (ocb + 1) * 128, c * ROWS : (c + 1) * ROWS, :],
                                in_=out_c[:],
                            )

            # interleave: image n+1's loads/signs are emitted (and thus
            # prioritized) ahead of image n's compute, so ACT/DVE run them
            # before that image's PSUM drains.  Image 0's first subgroup
            # (chunks 0-2) only reads padded rows <= 26, covered by the
            # first half-image signs; image 3 subgrouped for earlier tail
            # drains.
            load_image(1)
            compute_image(0, subs=((0, 3), (3, NCHUNK)))
            load_image(2)
            compute_image(1)
            load_image(3)
            compute_image(2)
            compute_image(3, subs=((0, 3), (3, NCHUNK)))

    _split_excess_waits(nc)
    return nc


def _get_nc():
    if "nc" not in _cache:
        _cache["nc"] = build_nc()
    return _cache["nc"]


def run(inputs, trace=False, trace_cores=None):
    from concourse.bass_utils import run_bass_kernel_spmd

    x = np.asarray(inputs["x"])
    weight = np.asarray(inputs["weight"])
    scale = np.asarray(inputs["scale"])

    # (kh, kw, ic, oc) layout so each tap's [ic, oc] block is contiguous
    wt = np.ascontiguousarray(weight.transpose(2, 3, 1, 0)).astype(np.float32)

    in_maps = [
        {"x": x[i * IMGS : (i + 1) * IMGS], "wt": wt, "scale": scale}
        for i in range(N_CORES)
    ]
    res = run_bass_kernel_spmd(
        _get_nc(),
        in_maps,
        core_ids=list(range(N_CORES)),
        trace=trace,
        trace_cores=trace_cores,
    )
    out = np.concatenate([res.results[i]["y"] for i in range(N_CORES)], axis=0)
    return out, res


def kernel(**inputs):
    # One retry: a previously crashed process can leave a core wedged
    # (NRT_EXEC_UNIT_UNRECOVERABLE); the runtime recovers on the next
    # attempt.
    try:
        out, _ = run(inputs, trace=False)
    except Exception:
        out, _ = run(inputs, trace=False)
    return out


# revision 29
# speedup vs baseline: 1.7922x; 1.0052x over previous
"""Binary conv2d (XNOR-style) + per-channel scale for Trainium2.

y = conv2d(sign(x), sign(w), stride=1, pad=1) * scale[oc]

Data-parallel over batch across 8 NeuronCores (4 images each).  Per
core: binarize x into a zero-padded 58x58 fp8/bf16 image laid out
[ic_part=128, slot, spatial] (slot = image*2 + ic_block); the 3x3 conv
over 256 in-channels becomes accumulating matmuls into a PSUM tile per
8-output-row chunk, using shifted windows of the padded image as the
moving operand.  With fp8 DoubleRow both ic blocks contract in one
matmul (K=256).  All matmul inputs are exactly +-1/0 (representable in
fp8/bf16) and PSUM accumulates in fp32, so the conv result is the exact
integer; the per-channel scale is applied in fp32 during the PSUM->SBUF
copy, making the result bit-identical to the fp32 reference.
"""

import numpy as np

N_CORES = 8
IMGS = 4  # images per core
IC = 256
OC = 256
H = W = 56
# Padded row stride is 57, not 58: for a 3-wide kernel the left pad of
# row r+1 doubles as the right pad of row r, halving the dead columns.
WPAD = 57
XPAD_F = 3312  # 58 padded rows * 57 = 3306 -> pad to mult of 16
ROWS = 8  # output rows per PSUM tile
NFREE = ROWS * WPAD  # 456 <= 512 (PSUM bank limit)
NCHUNK = H // ROWS  # 7

USE_FP8 = True

_cache = {}


def _install_drain_patch():
    """This walrus build rejects >1 sync-wait on ctrl-type instructions;
    Tile's kernel-tail drain carries one wait per pending proc.  Split it
    into one drain per proc (each with <=1 wait)."""
    import concourse.tile as _tile
    from concourse.vector_clock import ScopedClock, VectorClock

    if getattr(_tile.TileContext, "_drain_split_patch", False):
        return

    def _drain_and_barrier(self, tick_clock, wait_clock):
        # Drains AND the sem clears both run on the Pool engine (the
        # framework's clear_and_free_semaphores emits gpsimd dma_reset/
        # sem_clear), so the first all-engine barrier of the stock tail is
        # unnecessary: once Pool's drains observe every proc's final tick,
        # all sem increments have retired and Pool may clear immediately.
        nc = self.nc
        gclock = tick_clock.global_clock
        n = len(gclock)
        for p in range(n):
            t = gclock[p]
            if t <= 0:
                continue
            vec = [0] * n
            vec[p] = t
            d = nc.gpsimd.drain()
            wait_clock.add_sem_waits(d.ins, ScopedClock({None: VectorClock(vec)}))
        assert self.sems is not None
        popped = nc._tile_sem_poison_stack.pop()
        assert popped is self._sem_poison
        nc.clear_and_free_semaphores(list(self.sems.allocated().values()))
        # No final all-engine barrier: every other engine's stream simply
        # ends, NRT completion waits for all queues anyway, and the next
        # invocation cannot start before this one fully retires.

    _tile.TileContext._drain_and_barrier = _drain_and_barrier
    _tile.TileContext._drain_split_patch = True


def _split_excess_waits(nc, maxw=1):
    """Same walrus limitation: hoist excess sync-waits onto same-engine
    NoOps inserted just before the instruction (engine streams are
    in-order, so a preceding NoOp carrying the waits is equivalent)."""
    import concourse.mybir as mybir

    n_split = 0
    for f in nc.m.functions:
        for bb in f.blocks:
            out = []
            for ins in bb.instructions:
                si = ins.sync_info
                if si and si.on_wait and len(si.on_wait) > maxw:
                    waits = list(si.on_wait)
                    excess, keep = waits[:-maxw], waits[-maxw:]
                    for i in range(0, len(excess), maxw):
                        nop = mybir.InstNoOp(
                            name=f"{ins.name}_waitsplit{i}",
                            engine=ins.engine,
                            ins=[],
                            outs=[],
                            sync_info=mybir.SyncInfo(
                                on_wait=excess[i : i + maxw], on_update=[]
                            ),
                        )
                        out.append(nop)
                    si.on_wait = keep
                    n_split += 1
                out.append(ins)
            bb.instructions = out
    return n_split


def build_nc():
    import concourse.bass as bass
    import concourse.mybir as mybir
    from concourse.tile import TileContext

    _install_drain_patch()

    f32 = mybir.dt.float32
    bdt = mybir.dt.float8e4 if USE_FP8 else mybir.dt.bfloat16
    Copy = mybir.ActivationFunctionType.Copy

    nc = bass.Bass()
    x = nc.declare_dram_parameter("x", [IMGS, IC, H, W], f32, isOutput=False)
    wt = nc.declare_dram_parameter("wt", [3, 3, IC, OC], f32, isOutput=False)
    scale = nc.declare_dram_parameter("scale", [OC], f32, isOutput=False)
    y = nc.declare_dram_parameter("y", [IMGS, OC, H, W], f32, isOutput=True)

    with TileContext(nc) as tc:
        with (
            tc.tile_pool(name="const", bufs=1) as cpool,
            tc.tile_pool(name="xin", bufs=3) as xin_pool,
            tc.tile_pool(name="outp", bufs=4) as out_pool,
            tc.tile_pool(name="psum", bufs=8, space="PSUM") as psum_pool,
        ):
            # --- DMA issue serializes ~0.6us each on the SP queue and HBM
            # delivery follows issue order.  The t=0 weight tap goes first
            # (tiny, unblocks the first LDWEIGHTS), then the image-0 loads
            # that gate the first matmul, then the remaining taps.
            # NOTE: a DoubleRow rhs AP spans both ic slots of an image, so
            # its dep range covers the WHOLE slot pair -- partial-image
            # binarization cannot unlock any matmul; sign whole images.
            wf = cpool.tile([128, 18, OC], f32)
            wb = cpool.tile([128, 18, OC], bdt)
            wsrc = wt.rearrange("a b (i p) f -> p (a b i) f", p=128)

            nc.sync.dma_start(out=wf[:, 0:2, :], in_=wsrc[:, 0:2, :])

            # x0 on the SP HWDGE ring, x1 on the Scalar-engine ring: rings
            # are FIFO per issuing engine, so this parallelizes the two
            # transfers that gate the first matmul.
            xp = cpool.tile([128, IMGS * 2, XPAD_F], bdt)
            xins = {}
            for j, eng in ((0, nc.sync), (1, nc.scalar)):
                xin = xin_pool.tile([128, H, W], f32, name=f"xin{j}", tag="xin")
                eng.dma_start(out=xin[:], in_=x[0, j * 128 : (j + 1) * 128, :, :])
                xins[j] = xin

            for t in range(1, 9):
                nc.sync.dma_start(
                    out=wf[:, 2 * t : 2 * t + 2, :], in_=wsrc[:, 2 * t : 2 * t + 2, :]
                )
            sc = cpool.tile([128, 2], f32)
            nc.sync.dma_start(out=sc[:], in_=scale.rearrange("(b p) -> p b", p=128))

            def pad_ring(j):
                # zero only the padding ring (interior is overwritten by
                # the sign): top pad row; each data row's col 0 (which is
                # also the previous row's right pad); bottom pad row+tail.
                xpj = xp[:, j, :]
                nc.vector.memset(xpj[:, 0:WPAD], 0.0)
                lefts = xpj[:, WPAD : WPAD + H * WPAD].rearrange(
                    "p (r c) -> p r c", c=WPAD
                )[:, :, 0:1]
                nc.vector.memset(lefts, 0.0)
                nc.vector.memset(xpj[:, (H + 1) * WPAD : XPAD_F], 0.0)

            def sign_dst(j, r0=0, r1=H):
                # destination for x rows [r0, r1) = padded rows [r0+1, r1+1)
                base = (r0 + 1) * WPAD + 1
                return (
                    xp[:, j, base : base + (r1 - r0) * WPAD]
                    .rearrange("p (h w) -> p h w", w=WPAD)[:, :, 0:W]
                )

            def dve_sign(j, xin, r0=0, r1=H, tg="tmp"):
                # sign() on the vector engine: clamp(x * 1e38, -1, 1).
                # Exact for fp32 normals (|x|*1e38 saturates past +-1) and
                # for +-0; frees ScalarE, which owns the other signs.
                tmp = xin_pool.tile(
                    [128, r1 - r0, W], f32, name=f"tmp{j}_{r0}", tag=tg
                )
                nc.vector.tensor_scalar(
                    tmp[:], xin[:], 1.0e38, -1.0,
                    op0=mybir.AluOpType.mult, op1=mybir.AluOpType.max,
                )
                nc.vector.tensor_scalar_min(sign_dst(j, r0, r1), tmp[:], 1.0)

            pad_ring(0)
            pad_ring(1)
            nc.scalar.sign(wb[:, 0:2, :], wf[:, 0:2, :])  # t=0 taps, early
            nc.scalar.sign(sign_dst(0), xins[0][:])  # ACT
            dve_sign(1, xins[1])  # DVE, parallel with ACT
            for t in range(1, 9):
                nc.scalar.sign(wb[:, 2 * t : 2 * t + 2, :], wf[:, 2 * t : 2 * t + 2, :])

            def load_image(n):
                # input loads ride the Scalar-engine HWDGE ring, leaving
                # the SP ring to the (larger) output-store stream.
                for icb in range(2):
                    j = n * 2 + icb
                    xin = xin_pool.tile([128, H, W], f32, name=f"xin{j}", tag="xin")
                    nc.scalar.dma_start(
                        out=xin[:], in_=x[n, icb * 128 : (icb + 1) * 128, :, :]
                    )
                    pad_ring(j)
                    if icb == 0:
                        nc.scalar.sign(sign_dst(j), xin[:])
                    else:
                        dve_sign(j, xin)

            def compute_image(n, subs=((0, NCHUNK),)):
                # tap-outer (weight-stationary) so consecutive matmuls hit
                # different PSUM banks (same-bank back-to-back accumulation
                # serializes the drain/fill overlap).  LDWEIGHTS overlaps
                # MATMUL via the PE dual weight buffer.  `subs` splits the
                # chunk range so the first subgroup can start before the
                # whole image is binarized (n=0) / drain earlier (n=3).
                for c0, c1 in subs:
                    for ocb in range(2):
                        psums = [
                            psum_pool.tile(
                                [128, NFREE], f32, name=f"ps{n}{ocb}{c}", tag="ps"
                            )
                            for c in range(c0, c1)
                        ]
                        for t in range(9):
                            kh, kw = divmod(t, 3)
                            if USE_FP8:
                                lhsT = wb[:, 2 * t : 2 * t + 2, ocb * 128 : (ocb + 1) * 128]
                                rhs_slot = xp[:, 2 * n : 2 * n + 2, :]
                                for c in range(c0, c1):
                                    off = c * ROWS * WPAD + kh * WPAD + kw
                                    nc.tensor.matmul(
                                        psums[c - c0][:],
                                        lhsT,
                                        rhs_slot[:, :, off : off + NFREE],
                                        start=(t == 0),
                                        stop=(t == 8),
                                        perf_mode=mybir.MatmulPerfMode.DoubleRow,
                                    )
                            else:
                                for icb in range(2):
                                    for c in range(c0, c1):
                                        off = c * ROWS * WPAD + kh * WPAD + kw
                                        nc.tensor.matmul(
                                            psums[c - c0][:],
                                            wb[:, 2 * t + icb, ocb * 128 : (ocb + 1) * 128],
                                            xp[:, n * 2 + icb, off : off + NFREE],
                                            start=(t == 0 and icb == 0),
                                            stop=(t == 8 and icb == 1),
                                        )
                        for c in range(c0, c1):
                            out_c = out_pool.tile([128, ROWS, W], f32)
                            src = psums[c - c0].rearrange("p (h w) -> p h w", w=WPAD)[
                                :, :, 0:W
                            ]
                            # alternate drain engine; both apply fp32 scale
                            if c % 2 == 1:
                                nc.scalar.activation(
                                    out_c[:], src, Copy, scale=sc[:, ocb : ocb + 1]
                                )
                            else:
                                nc.vector.tensor_scalar_mul(
                                    out_c[:], src, sc[:, ocb : ocb + 1]
                                )
                            nc.sync.dma_start(
                                out=y[n, ocb * 128 : (ocb + 1) * 128, c * ROWS : (c + 1) * ROWS, :],
                                in_=out_c[:],
                            )

            # interleave: image n+1's loads/signs are emitted (and thus
            # prioritized) ahead of image n's compute, so ACT/DVE run them
            # before that image's PSUM drains.  Image 0's first subgroup
            # (chunks 0-2) only reads padded rows <= 26, covered by the
            # first half-image signs; image 3 subgrouped for earlier tail
            # drains.
            load_image(1)
            compute_image(0, subs=((0, 3), (3, NCHUNK)))
            load_image(2)
            compute_image(1)
            load_image(3)
            compute_image(2)
            compute_image(3, subs=((0, 3), (3, NCHUNK)))

    _split_excess_waits(nc)
    return nc


def _get_nc():
    if "nc" not in _cache:
        _cache["nc"] = build_nc()
    return _cache["nc"]


def run(inputs, trace=False, trace_cores=None):
    from concourse.bass_utils import run_bass_kernel_spmd

    x = np.asarray(inputs["x"])
    weight = np.asarray(inputs["weight"])
    scale = np.asarray(inputs["scale"])

    # (kh, kw, ic, oc) layout so each tap's [ic, oc] block is contiguous
    wt = np.ascontiguousarray(weight.transpose(2, 3, 1, 0)).astype(np.float32)

    in_maps = [
        {"x": x[i * IMGS : (i + 1) * IMGS], "wt": wt, "scale": scale}
        for i in range(N_CORES)
    ]
    res = run_bass_kernel_spmd(
        _get_nc(),
        in_maps,
        core_ids=list(range(N_CORES)),
        trace=trace,
        trace_cores=trace_cores,
    )
    out = np.concatenate([res.results[i]["y"] for i in range(N_CORES)], axis=0)
    return out, res


def kernel(**inputs):
    # One retry: a previously crashed process can leave a core wedged
    # (NRT_EXEC_UNIT_UNRECOVERABLE); the runtime recovers on the next
    # attempt.
    try:
        out, _ = run(inputs, trace=False)
    except Exception:
        out, _ = run(inputs, trace=False)
    return out


# revision 31
# speedup vs baseline: 1.7985x; 1.0035x over previous
"""Binary conv2d (XNOR-style) + per-channel scale for Trainium2.

y = conv2d(sign(x), sign(w), stride=1, pad=1) * scale[oc]

Data-parallel over batch across 8 NeuronCores (4 images each).  Per
core: binarize x into a zero-padded 58-row x 57-stride fp8/bf16 image
laid out [ic_part=128, slot, spatial] (slot = image*2 + ic_block); the
3x3 conv over 256 in-channels becomes accumulating matmuls into a PSUM
tile per 8-output-row chunk, using shifted windows of the padded image
as the moving operand.  With fp8 DoubleRow both ic blocks contract in one
matmul (K=256).  All matmul inputs are exactly +-1/0 (representable in
fp8/bf16) and PSUM accumulates in fp32, so the conv result is the exact
integer; the per-channel scale is applied in fp32 during the PSUM->SBUF
copy, making the result bit-identical to the fp32 reference.
"""

import numpy as np

N_CORES = 8
IMGS = 4  # images per core
IC = 256
OC = 256
H = W = 56
# Padded row stride is 57, not 58: for a 3-wide kernel the left pad of
# row r+1 doubles as the right pad of row r, halving the dead columns.
WPAD = 57
XPAD_F = 3312  # 58 padded rows * 57 = 3306 -> pad to mult of 16
ROWS = 8  # output rows per PSUM tile
NFREE = ROWS * WPAD  # 456 <= 512 (PSUM bank limit)
NCHUNK = H // ROWS  # 7

USE_FP8 = True

_cache = {}


def _install_drain_patch():
    """This walrus build rejects >1 sync-wait on ctrl-type instructions;
    Tile's kernel-tail drain carries one wait per pending proc.  Split it
    into one drain per proc (each with <=1 wait)."""
    import concourse.tile as _tile
    from concourse.vector_clock import ScopedClock, VectorClock

    if getattr(_tile.TileContext, "_drain_split_patch", False):
        return

    def _drain_and_barrier(self, tick_clock, wait_clock):
        # Drains AND the sem clears both run on the Pool engine (the
        # framework's clear_and_free_semaphores emits gpsimd dma_reset/
        # sem_clear), so the first all-engine barrier of the stock tail is
        # unnecessary: once Pool's drains observe every proc's final tick,
        # all sem increments have retired and Pool may clear immediately.
        nc = self.nc
        gclock = tick_clock.global_clock
        n = len(gclock)
        for p in range(n):
            t = gclock[p]
            if t <= 0:
                continue
            vec = [0] * n
            vec[p] = t
            d = nc.gpsimd.drain()
            wait_clock.add_sem_waits(d.ins, ScopedClock({None: VectorClock(vec)}))
        assert self.sems is not None
        popped = nc._tile_sem_poison_stack.pop()
        assert popped is self._sem_poison
        nc.clear_and_free_semaphores(list(self.sems.allocated().values()))
        # No final all-engine barrier: every other engine's stream simply
        # ends, NRT completion waits for all queues anyway, and the next
        # invocation cannot start before this one fully retires.

    _tile.TileContext._drain_and_barrier = _drain_and_barrier
    _tile.TileContext._drain_split_patch = True


def _split_excess_waits(nc, maxw=1):
    """Same walrus limitation: hoist excess sync-waits onto same-engine
    NoOps inserted just before the instruction (engine streams are
    in-order, so a preceding NoOp carrying the waits is equivalent)."""
    import concourse.mybir as mybir

    n_split = 0
    for f in nc.m.functions:
        for bb in f.blocks:
            out = []
            for ins in bb.instructions:
                si = ins.sync_info
                if si and si.on_wait and len(si.on_wait) > maxw:
                    waits = list(si.on_wait)
                    excess, keep = waits[:-maxw], waits[-maxw:]
                    for i in range(0, len(excess), maxw):
                        nop = mybir.InstNoOp(
                            name=f"{ins.name}_waitsplit{i}",
                            engine=ins.engine,
                            ins=[],
                            outs=[],
                            sync_info=mybir.SyncInfo(
                                on_wait=excess[i : i + maxw], on_update=[]
                            ),
                        )
                        out.append(nop)
                    si.on_wait = keep
                    n_split += 1
                out.append(ins)
            bb.instructions = out
    return n_split


def build_nc():
    import concourse.bass as bass
    import concourse.mybir as mybir
    from concourse.tile import TileContext

    _install_drain_patch()

    f32 = mybir.dt.float32
    bdt = mybir.dt.float8e4 if USE_FP8 else mybir.dt.bfloat16
    Copy = mybir.ActivationFunctionType.Copy

    nc = bass.Bass()
    x = nc.declare_dram_parameter("x", [IMGS, IC, H, W], f32, isOutput=False)
    wt = nc.declare_dram_parameter("wt", [3, 3, IC, OC], f32, isOutput=False)
    scale = nc.declare_dram_parameter("scale", [OC], f32, isOutput=False)
    y = nc.declare_dram_parameter("y", [IMGS, OC, H, W], f32, isOutput=True)

    with TileContext(nc) as tc:
        with (
            tc.tile_pool(name="const", bufs=1) as cpool,
            tc.tile_pool(name="xin", bufs=3) as xin_pool,
            tc.tile_pool(name="outp", bufs=4) as out_pool,
            tc.tile_pool(name="psum", bufs=8, space="PSUM") as psum_pool,
        ):
            # --- DMA issue serializes ~0.6us each on the SP queue and HBM
            # delivery follows issue order.  The t=0 weight tap goes first
            # (tiny, unblocks the first LDWEIGHTS), then the image-0 loads
            # that gate the first matmul, then the remaining taps.
            # NOTE: a DoubleRow rhs AP spans both ic slots of an image, so
            # its dep range covers the WHOLE slot pair -- partial-image
            # binarization cannot unlock any matmul; sign whole images.
            wf = cpool.tile([128, 18, OC], f32)
            wb = cpool.tile([128, 18, OC], bdt)
            wsrc = wt.rearrange("a b (i p) f -> p (a b i) f", p=128)

            nc.sync.dma_start(out=wf[:, 0:2, :], in_=wsrc[:, 0:2, :])

            # x0 on the SP HWDGE ring, x1 on the Scalar-engine ring: rings
            # are FIFO per issuing engine, so this parallelizes the two
            # transfers that gate the first matmul.
            xp = cpool.tile([128, IMGS * 2, XPAD_F], bdt)
            xins = {}
            for j, eng in ((0, nc.sync), (1, nc.scalar)):
                xin = xin_pool.tile([128, H, W], f32, name=f"xin{j}", tag="xin")
                eng.dma_start(out=xin[:], in_=x[0, j * 128 : (j + 1) * 128, :, :])
                xins[j] = xin

            for t in range(1, 9):
                nc.sync.dma_start(
                    out=wf[:, 2 * t : 2 * t + 2, :], in_=wsrc[:, 2 * t : 2 * t + 2, :]
                )
            sc = cpool.tile([128, 2], f32)
            nc.sync.dma_start(out=sc[:], in_=scale.rearrange("(b p) -> p b", p=128))

            def pad_ring(j):
                # zero only the padding ring (interior is overwritten by
                # the sign): top pad row; each data row's col 0 (which is
                # also the previous row's right pad); bottom pad row+tail.
                xpj = xp[:, j, :]
                nc.vector.memset(xpj[:, 0:WPAD], 0.0)
                lefts = xpj[:, WPAD : WPAD + H * WPAD].rearrange(
                    "p (r c) -> p r c", c=WPAD
                )[:, :, 0:1]
                nc.vector.memset(lefts, 0.0)
                nc.vector.memset(xpj[:, (H + 1) * WPAD : XPAD_F], 0.0)

            def sign_dst(j, r0=0, r1=H):
                # destination for x rows [r0, r1) = padded rows [r0+1, r1+1)
                base = (r0 + 1) * WPAD + 1
                return (
                    xp[:, j, base : base + (r1 - r0) * WPAD]
                    .rearrange("p (h w) -> p h w", w=WPAD)[:, :, 0:W]
                )

            def dve_sign(j, xin, r0=0, r1=H, tg="tmp"):
                # sign() on the vector engine: clamp(x * 1e38, -1, 1).
                # Exact for fp32 normals (|x|*1e38 saturates past +-1) and
                # for +-0; frees ScalarE, which owns the other signs.
                tmp = xin_pool.tile(
                    [128, r1 - r0, W], f32, name=f"tmp{j}_{r0}", tag=tg
                )
                nc.vector.tensor_scalar(
                    tmp[:], xin[:], 1.0e38, -1.0,
                    op0=mybir.AluOpType.mult, op1=mybir.AluOpType.max,
                )
                nc.vector.tensor_scalar_min(sign_dst(j, r0, r1), tmp[:], 1.0)

            pad_ring(0)
            pad_ring(1)
            nc.scalar.sign(wb[:, 0:2, :], wf[:, 0:2, :])  # t=0 taps, early
            nc.scalar.sign(sign_dst(0), xins[0][:])  # ACT
            dve_sign(1, xins[1])  # DVE, parallel with ACT
            for t in range(1, 9):
                nc.scalar.sign(wb[:, 2 * t : 2 * t + 2, :], wf[:, 2 * t : 2 * t + 2, :])

            def load_image(n):
                # input loads ride the Scalar-engine HWDGE ring, leaving
                # the SP ring to the (larger) output-store stream.
                for icb in range(2):
                    j = n * 2 + icb
                    xin = xin_pool.tile([128, H, W], f32, name=f"xin{j}", tag="xin")
                    nc.scalar.dma_start(
                        out=xin[:], in_=x[n, icb * 128 : (icb + 1) * 128, :, :]
                    )
                    pad_ring(j)
                    if icb == 0:
                        nc.scalar.sign(sign_dst(j), xin[:])
                    else:
                        dve_sign(j, xin)

            def compute_image(n, subs=((0, NCHUNK),)):
                # tap-outer (weight-stationary) so consecutive matmuls hit
                # different PSUM banks (same-bank back-to-back accumulation
                # serializes the drain/fill overlap).  LDWEIGHTS overlaps
                # MATMUL via the PE dual weight buffer.  `subs` splits the
                # chunk range so the first subgroup can start before the
                # whole image is binarized (n=0) / drain earlier (n=3).
                for c0, c1 in subs:
                    for ocb in range(2):
                        psums = [
                            psum_pool.tile(
                                [128, NFREE], f32, name=f"ps{n}{ocb}{c}", tag="ps"
                            )
                            for c in range(c0, c1)
                        ]
                        for t in range(9):
                            kh, kw = divmod(t, 3)
                            if USE_FP8:
                                lhsT = wb[:, 2 * t : 2 * t + 2, ocb * 128 : (ocb + 1) * 128]
                                rhs_slot = xp[:, 2 * n : 2 * n + 2, :]
                                for c in range(c0, c1):
                                    off = c * ROWS * WPAD + kh * WPAD + kw
                                    nc.tensor.matmul(
                                        psums[c - c0][:],
                                        lhsT,
                                        rhs_slot[:, :, off : off + NFREE],
                                        start=(t == 0),
                                        stop=(t == 8),
                                        perf_mode=mybir.MatmulPerfMode.DoubleRow,
                                    )
                            else:
                                for icb in range(2):
                                    for c in range(c0, c1):
                                        off = c * ROWS * WPAD + kh * WPAD + kw
                                        nc.tensor.matmul(
                                            psums[c - c0][:],
                                            wb[:, 2 * t + icb, ocb * 128 : (ocb + 1) * 128],
                                            xp[:, n * 2 + icb, off : off + NFREE],
                                            start=(t == 0 and icb == 0),
                                            stop=(t == 8 and icb == 1),
                                        )
                        for c in range(c0, c1):
                            out_c = out_pool.tile([128, ROWS, W], f32)
                            src = psums[c - c0].rearrange("p (h w) -> p h w", w=WPAD)[
                                :, :, 0:W
                            ]
                            # alternate drain engine; both apply fp32 scale
                            if c % 2 == 1:
                                nc.scalar.activation(
                                    out_c[:], src, Copy, scale=sc[:, ocb : ocb + 1]
                                )
                            else:
                                nc.vector.tensor_scalar_mul(
                                    out_c[:], src, sc[:, ocb : ocb + 1]
                                )
                            nc.sync.dma_start(
                                out=y[n, ocb * 128 : (ocb + 1) * 128, c * ROWS : (c + 1) * ROWS, :],
                                in_=out_c[:],
                            )

            # interleave: image n+1's loads/signs are emitted (and thus
            # prioritized) ahead of image n's compute, so ACT/DVE run them
            # before that image's PSUM drains.  Images 0 and 3 are chunk-
            # subgrouped so their PSUM drains start/finish earlier at the
            # kernel boundaries.
            load_image(1)
            compute_image(0, subs=((0, 3), (3, NCHUNK)))
            load_image(2)
            compute_image(1)
            load_image(3)
            compute_image(2)
            compute_image(3, subs=((0, 3), (3, NCHUNK)))

    _split_excess_waits(nc)
    return nc


def _get_nc():
    if "nc" not in _cache:
        _cache["nc"] = build_nc()
    return _cache["nc"]


def run(inputs, trace=False, trace_cores=None):
    from concourse.bass_utils import run_bass_kernel_spmd

    x = np.asarray(inputs["x"])
    weight = np.asarray(inputs["weight"])
    scale = np.asarray(inputs["scale"])

    # (kh, kw, ic, oc) layout so each tap's [ic, oc] block is contiguous
    wt = np.ascontiguousarray(weight.transpose(2, 3, 1, 0)).astype(np.float32)

    in_maps = [
        {"x": x[i * IMGS : (i + 1) * IMGS], "wt": wt, "scale": scale}
        for i in range(N_CORES)
    ]
    res = run_bass_kernel_spmd(
        _get_nc(),
        in_maps,
        core_ids=list(range(N_CORES)),
        trace=trace,
        trace_cores=trace_cores,
    )
    out = np.concatenate([res.results[i]["y"] for i in range(N_CORES)], axis=0)
    return out, res


def kernel(**inputs):
    # One retry: a previously crashed process can leave a core wedged
    # (NRT_EXEC_UNIT_UNRECOVERABLE); the runtime recovers on the next
    # attempt.
    try:
        out, _ = run(inputs, trace=False)
    except Exception:
        out, _ = run(inputs, trace=False)
    return out
